# revision 28
# baseline (speedup 1.0000x reference)
"""MoEXLayer forward on 8 Trainium2 NeuronCores.

Math (reference, eval mode):
  W_rec[e] = W*alpha[e] + beta[e];  mu_w = mean_h(W_rec);  var_w = var_h(W_rec)
  Since alpha/beta are constant over h:
     mu_w[e,d]  = Wbar[d]*alpha[e,d] + beta[e,d],   Wbar = mean_h W
     var_w[e,d] = Vw[d]*alpha[e,d]^2,               Vw   = var_h W   (ddof=0)
  mu  = x @ mu_w.T + mean(bias); sig = sqrt(x^2 @ var_w.T + 1e-8)
  logits = erf(mu / (sqrt2*sig)); top-2 softmax -> router weights w1,w2
  out = sum_k w_k * relu(x @ (W*alpha[e_k]).T + bias)

Device strategy (data parallel over tokens, 512 tokens/core):
  - Router matmuls in fp32 (tiny: [128,1024]x[1024,8]).
  - Top-2 per token via the DVE Max8 sort; one-hots via is_equal vs v1/v2.
  - Per-token selected alpha row built with a tiny matmul over the 8-wide
    expert axis: A_k^T = alpha^T @ onehot_k^T, then xe = x * A_k (so only
    K=2 of the E=8 expert GEMMs are ever computed; no gather/scatter).
  - Main GEMMs in bf16 at N=512 against streamed W^T blocks; bias is added
    by a rank-1 ones x bias matmul into the same PSUM accumulation group;
    relu and the router weight are fused on the Scalar engine using
    relu(w*p) = w*relu(p) (w>0 as a softmax weight).
"""

import numpy as np
from contextlib import ExitStack

import os
import sys

if "/opt/trn_rl_repo" not in sys.path:
    sys.path.insert(0, "/opt/trn_rl_repo")

import ml_dtypes
import concourse.bass as bass
import concourse.tile as tile
from concourse import bacc, mybir
from concourse.bass_utils import run_bass_kernel_spmd

FP32 = mybir.dt.float32
BF16 = mybir.dt.bfloat16
AF = mybir.ActivationFunctionType
ALU = mybir.AluOpType

B, S, D, H, E = 2, 2048, 1024, 4096, 8
NCORES = 8
T = (B * S) // NCORES          # 512 tokens per core
NT = T // 128                  # 4 token tiles per core
DC = D // 128                  # 8 contraction chunks
HC = H // 512                  # 8 output column chunks
TG = T // 256                  # 2 selection token groups (N=256 keeps PE fast)


def _emit(ctx: ExitStack, tc: tile.TileContext, io: dict):
    nc = tc.nc
    xt, wt, alpha = io["xt"], io["wt"], io["alpha"]
    muw, varw = io["muw"], io["varw"]
    mb, nbias = io["mb"], io["nbias"]
    out = io["out"]

    const = ctx.enter_context(tc.tile_pool(name="const", bufs=1))
    persist = ctx.enter_context(tc.tile_pool(name="persist", bufs=1))

    # ---- small constant/parameter tiles ----
    muw_sb = const.tile([128, E * DC], FP32, name="muw_sb")
    varw_sb = const.tile([128, E * DC], FP32, name="varw_sb")
    mb_sb = const.tile([128, 1], FP32, name="mb_sb")
    alpha_sb = const.tile([E, D], BF16, name="alpha_sb")
    nbias_sb = const.tile([1, H], BF16, name="nbias_sb")
    ones_sb = const.tile([1, 128], BF16, name="ones_sb")
    ident_sb = const.tile([128, 128], FP32, name="ident_sb")
    eps_sb = const.tile([128, 1], FP32, name="eps_sb")
    nc.vector.memset(eps_sb[:], 2e-8)

    nc.sync.dma_start(muw_sb[:], muw[:])
    nc.sync.dma_start(varw_sb[:], varw[:])
    nc.sync.dma_start(mb_sb[:], mb[:])
    nc.sync.dma_start(alpha_sb[:], alpha[:])
    nc.sync.dma_start(nbias_sb[:], nbias[:])
    nc.vector.memset(ones_sb[:], 1.0)
    # identity for PE-transpose: keep ones where (p - f) == 0
    nc.vector.memset(ident_sb[:], 1.0)
    nc.gpsimd.affine_select(
        ident_sb[:], ident_sb[:], pattern=[[-1, 128]], base=0,
        channel_multiplier=1, compare_op=ALU.is_equal, fill=0.0,
    )

    # ---- x^T tiles (each a fully contiguous 256KB DRAM block) + x^2 ----
    xt_sb = []
    x2_sb = []
    for c in range(DC):
        t_ = persist.tile([128, T], FP32, name=f"xt{c}", tag=f"xt{c}")
        nc.sync.dma_start(t_[:], xt[128 * c:128 * (c + 1), :])
        xt_sb.append(t_)
    for c in range(DC):
        t_ = persist.tile([128, T], FP32, name=f"x2{c}", tag=f"x2{c}")
        nc.scalar.activation(t_[:], xt_sb[c][:], AF.Square)
        x2_sb.append(t_)

    mu_w = [muw_sb[:, E * c:E * (c + 1)] for c in range(DC)]
    var_w = [varw_sb[:, E * c:E * (c + 1)] for c in range(DC)]

    # ---- W^T fully resident: 8 big tiles [128, 4096] bf16 (8KB rows keep
    # the DMA descriptor count low; matmuls slice out [128, 512] columns) ----
    wt_sb = []
    for c in range(DC):
        w_ = persist.tile([128, H], BF16, name=f"wt{c}", tag=f"wt{c}")
        nc.sync.dma_start(w_[:], wt[128 * c:128 * (c + 1), :])
        wt_sb.append(w_)

    # ---- router + selection in two 256-token halves: the second half's
    # matmuls fill the first half's scalar-chain PE gap, and main GEMMs of
    # half 0 fill half 1's chain gap ----
    GT = 256
    xe = [[[None] * TG for _ in range(DC)] for _ in range(2)]
    for k in range(2):
        for c in range(DC):
            for g in range(TG):
                xe[k][c][g] = persist.tile([128, GT], BF16, name=f"xe{k}_{c}_{g}",
                                           tag=f"xe{k}_{c}_{g}")
    w_all = [None] * NT
    nb_sb = []
    sbuf_out = ctx.enter_context(tc.tile_pool(name="sbuf_out", bufs=1))
    spb = ctx.enter_context(tc.tile_pool(name="spb", bufs=2, space="PSUM"))
    spa = ctx.enter_context(tc.tile_pool(name="spa", bufs=2, space="PSUM"))

    def emit_router(g):
        gsl = slice(GT * g, GT * (g + 1))
        muT = spa.tile([E, GT], FP32, name=f"muT{g}", tag="spa")
        for c in range(DC):
            nc.tensor.matmul(muT[:], lhsT=mu_w[c], rhs=xt_sb[c][:, gsl],
                             start=(c == 0), stop=(c == DC - 1))
        margT = persist.tile([E, GT], FP32, name=f"margT{g}", tag=f"margT{g}")
        nc.vector.tensor_scalar_add(margT[:], muT[:], mb_sb[0:E, 0:1])
        vaT = spa.tile([E, GT], FP32, name=f"vaT{g}", tag="spa")
        for c in range(DC):
            nc.tensor.matmul(vaT[:], lhsT=var_w[c], rhs=x2_sb[c][:, gsl],
                             start=(c == 0), stop=(c == DC - 1))
        if g == 0:
            # -bias rows broadcast across partitions (fills the chain gap)
            for j in range(HC):
                nps = spb.tile([128, 512], FP32, name=f"nb_ps{j}", tag="spb")
                nc.tensor.matmul(nps[:], lhsT=ones_sb[:],
                                 rhs=nbias_sb[:, 512 * j:512 * (j + 1)],
                                 start=True, stop=True)
                nb_ = persist.tile([128, 512], BF16, name=f"nb{j}", tag=f"nb{j}")
                nc.vector.tensor_copy(nb_[:], nps[:])
                nb_sb.append(nb_)
        # sqrt(2*var + 2e-8) = sqrt(2)*sigma
        sig2T = persist.tile([E, GT], FP32, name=f"sig2T{g}", tag=f"sig2T{g}")
        nc.scalar.activation(sig2T[:], vaT[:], AF.Sqrt, bias=eps_sb[0:E, 0:1],
                             scale=2.0)
        recT = persist.tile([E, GT], FP32, name=f"recT{g}", tag=f"recT{g}")
        rscr = persist.tile([E, GT], FP32, name=f"rscr{g}", tag=f"rscr{g}")
        nc.vector.reciprocal_approx_accurate(recT[:], sig2T[:], rscr[:])
        logT = persist.tile([E, GT], FP32, name=f"logT{g}", tag=f"logT{g}")
        nc.vector.tensor_tensor(logT[:], margT[:], recT[:], op=ALU.mult)
        nc.scalar.activation(logT[:], logT[:], AF.Erf)
        return logT

    def emit_topk_sel(g, logT):
        gsl = slice(GT * g, GT * (g + 1))
        ohT = [None, None]
        for k in range(2):
            ohT[k] = persist.tile([E, GT], BF16, name=f"ohT{k}_{g}",
                                  tag=f"ohT{k}_{g}")
        for hh in range(2):
            ti = 2 * g + hh
            hsl = slice(128 * hh, 128 * (hh + 1))
            lg_ps = spb.tile([128, E], FP32, name=f"lg_ps{ti}", tag="spb")
            nc.tensor.transpose(lg_ps[:], logT[:, hsl], ident_sb[0:E, 0:E])
            lg = persist.tile([128, E], FP32, name=f"lg{ti}", tag=f"lg{ti}")
            nc.vector.tensor_copy(lg[:], lg_ps[:])
            mx = persist.tile([128, 8], FP32, name=f"mx{ti}", tag=f"mx{ti}")
            nc.vector.max(mx[:], lg[:])
            o1 = persist.tile([128, E], FP32, name=f"oh1_{ti}", tag=f"oh1_{ti}")
            nc.vector.tensor_scalar(o1[:], lg[:], mx[:, 0:1], None,
                                    op0=ALU.is_equal)
            o2 = persist.tile([128, E], FP32, name=f"oh2_{ti}", tag=f"oh2_{ti}")
            nc.vector.tensor_scalar(o2[:], lg[:], mx[:, 1:2], None,
                                    op0=ALU.is_equal)
            d_ = persist.tile([128, 1], FP32, name=f"d21_{ti}", tag=f"d21_{ti}")
            nc.vector.tensor_tensor(d_[:], mx[:, 0:1], mx[:, 1:2],
                                    op=ALU.subtract)
            w_ = persist.tile([128, 2], FP32, name=f"w{ti}", tag=f"w{ti}")
            nc.scalar.activation(w_[:, 0:1], d_[:], AF.Sigmoid)
            nc.vector.tensor_scalar(w_[:, 1:2], w_[:, 0:1], -1.0, 1.0,
                                    op0=ALU.mult, op1=ALU.add)
            w_all[ti] = w_
            for k, o_ in ((0, o1), (1, o2)):
                tp = spb.tile([E, 128], FP32, name=f"ohTp{k}_{ti}", tag="spb")
                nc.tensor.transpose(tp[:], o_[:], ident_sb[:])
                nc.vector.tensor_copy(ohT[k][:, hsl], tp[:])
        # selection: xe[k][c][g] = x * alpha[e_k(t)]  (bf16, [d, t] layout)
        for c in range(DC):
            for k in range(2):
                a_ps = spb.tile([128, GT], FP32, name=f"a_ps{g}{k}{c}", tag="spb")
                nc.tensor.matmul(a_ps[:], lhsT=alpha_sb[:, 128 * c:128 * (c + 1)],
                                 rhs=ohT[k][:], start=True, stop=True)
                nc.vector.tensor_tensor(xe[k][c][g][:], xt_sb[c][:, gsl],
                                        a_ps[:], op=ALU.mult)

    # ---- main GEMMs: JQ h-chunks of 512 share one stationary load ----
    JQ = 2
    ps_main = ctx.enter_context(tc.tile_pool(name="ps_main", bufs=4, space="PSUM"))

    def emit_main(ti, jq, js=None):
        js = list(range(jq * JQ, (jq + 1) * JQ)) if js is None else js
        tsl = slice(128 * ti, 128 * (ti + 1))
        g, hh = ti // 2, ti % 2
        hsl = slice(128 * hh, 128 * (hh + 1))
        s_tiles = [[None] * len(js), [None] * len(js)]
        for k in range(2):
            ps = [ps_main.tile([128, 512], FP32, name=f"ps{jq}_{ti}_{k}_{jj}",
                               tag="ps_main") for jj in range(len(js))]
            for c in range(DC):
                for jj, j in enumerate(js):
                    nc.tensor.matmul(ps[jj][:], lhsT=xe[k][c][g][:, hsl],
                                     rhs=wt_sb[c][:, 512 * j:512 * (j + 1)],
                                     start=(c == 0), stop=(c == DC - 1))
            for jj, j in enumerate(js):
                # relu(p + b) == max(p, -b) + b; the +b lands after combine
                # (w1 + w2 == 1 exactly: w1 >= 0.5 so 1 - w1 is Sterbenz-exact)
                m_ = sbuf_out.tile([128, 512], FP32, name=f"m{jq}_{ti}_{k}_{jj}",
                                   tag=f"s{k}", bufs=JQ + 2)
                nc.vector.tensor_tensor(m_[:], ps[jj][:], nb_sb[j][:], op=ALU.max)
                # w_k * max(p, -b) on the Scalar engine (Copy with scale AP)
                nc.scalar.activation(m_[:], m_[:], AF.Copy,
                                     scale=w_all[ti][:, k:k + 1])
                s_tiles[k][jj] = m_
        o_ = sbuf_out.tile([128, 512 * len(js)], FP32, name=f"o{jq}_{ti}",
                           tag="otile", bufs=4)
        for jj, j in enumerate(js):
            u_ = sbuf_out.tile([128, 512], FP32, name=f"u{jq}_{ti}_{jj}",
                               tag="utile", bufs=4)
            nc.vector.tensor_tensor(u_[:], s_tiles[0][jj][:], s_tiles[1][jj][:],
                                    op=ALU.add)
            nc.vector.tensor_tensor(o_[:, 512 * jj:512 * (jj + 1)],
                                    u_[:], nb_sb[j][:], op=ALU.subtract)
        nc.sync.dma_start(out[tsl, 512 * js[0]:512 * (js[-1] + 1)], o_[:])

    # emission order: g0 routing, B(t0), then g1 routing (fills B(t0)'s
    # stalls and its scalar chain overlaps B), then the remaining tiles
    logT = emit_router(0)
    emit_topk_sel(0, logT)
    for jq in range(HC // JQ):
        emit_main(0, jq)
    logT = emit_router(1)
    emit_topk_sel(1, logT)
    for ti in range(1, NT):
        for jq in range(HC // JQ):
            if ti == NT - 1 and jq == HC // JQ - 1:
                # split the final group so its epilogue overlaps compute
                emit_main(ti, jq, js=[HC - 2])
                emit_main(ti, jq + 1, js=[HC - 1])
            else:
                emit_main(ti, jq)


_CACHE = {}

if os.environ.get("BASS_LDW_OPT") == "1":
    import concourse.bass_utils as _bu

    _orig_run_command = _bu.run_command

    def _run_command_ldw(cmd, *a, **kw):
        cmd = ["--enable-ldw-opt=true" if c == "--enable-ldw-opt=false" else c
               for c in cmd]
        return _orig_run_command(cmd, *a, **kw)

    _bu.run_command = _run_command_ldw


def _build():
    if "nc" in _CACHE:
        return _CACHE["nc"]
    nc = bacc.Bacc("TRN2", target_bir_lowering=False, debug=False,
                   num_devices=NCORES)
    io = {
        "xt": nc.dram_tensor("xt", [D, T], FP32, kind="ExternalInput").ap(),
        "wt": nc.dram_tensor("wt", [D, H], BF16, kind="ExternalInput").ap(),
        "alpha": nc.dram_tensor("alpha", [E, D], BF16, kind="ExternalInput").ap(),
        "muw": nc.dram_tensor("muw", [128, E * DC], FP32,
                              kind="ExternalInput").ap(),
        "varw": nc.dram_tensor("varw", [128, E * DC], FP32,
                               kind="ExternalInput").ap(),
        "mb": nc.dram_tensor("mb", [128, 1], FP32, kind="ExternalInput").ap(),
        "nbias": nc.dram_tensor("nbias", [1, H], BF16, kind="ExternalInput").ap(),
        "out": nc.dram_tensor("out", [T, H], FP32, kind="ExternalOutput").ap(),
    }
    with tile.TileContext(nc) as tc, ExitStack() as ctx:
        _emit(ctx, tc, io)
    nc.compile()
    _CACHE["nc"] = nc
    return nc


def _chunk_cols(m):
    # [D, n] -> [128, DC*n] where columns [n*c : n*(c+1)] hold rows 128c..128c+127
    n = m.shape[1]
    return np.ascontiguousarray(
        m.reshape(DC, 128, n).transpose(1, 0, 2).reshape(128, DC * n))


def make_in_maps(x, W, bias, alpha, beta):
    tokens = np.ascontiguousarray(x.reshape(B * S, D))
    Wbar = W.mean(axis=0).astype(np.float32)
    Vw = W.var(axis=0).astype(np.float32)
    mu_w = (Wbar[None, :] * alpha + beta).astype(np.float32)    # [E, D]
    var_w = (Vw[None, :] * alpha * alpha).astype(np.float32)    # [E, D]
    mb = np.full((128, 1), bias.mean(), dtype=np.float32)
    wt_bf = np.ascontiguousarray(W.T).astype(ml_dtypes.bfloat16)
    muw_c = _chunk_cols(np.ascontiguousarray(mu_w.T))
    varw_c = _chunk_cols(np.ascontiguousarray(var_w.T))
    nbias = (-bias).reshape(1, H).astype(ml_dtypes.bfloat16)
    common = dict(wt=wt_bf, alpha=np.ascontiguousarray(alpha).astype(ml_dtypes.bfloat16),
                  muw=muw_c, varw=varw_c, mb=mb, nbias=nbias)
    maps = []
    for m in range(NCORES):
        xs = np.ascontiguousarray(tokens[T * m:T * (m + 1)].T.astype(np.float32))
        maps.append(dict(xt=xs, **common))
    return maps


def run(x, W, bias, alpha, beta, trace=False, **kw):
    nc = _build()
    maps = make_in_maps(x, W, bias, alpha, beta)
    res = run_bass_kernel_spmd(nc, maps, core_ids=list(range(NCORES)),
                               trace=trace, **kw)
    outs = [res.results[m]["out"] for m in range(NCORES)]
    full = np.concatenate(outs, axis=0).reshape(B, S, H).astype(np.float32)
    return full, res


def kernel(x, W, bias, alpha, beta):
    full, _ = run(np.asarray(x), np.asarray(W), np.asarray(bias),
                  np.asarray(alpha), np.asarray(beta))
    return full


# revision 29
# speedup vs baseline: 1.0252x; 1.0252x over previous
"""MoEXLayer forward on 8 Trainium2 NeuronCores.

Math (reference, eval mode):
  W_rec[e] = W*alpha[e] + beta[e];  mu_w = mean_h(W_rec);  var_w = var_h(W_rec)
  Since alpha/beta are constant over h:
     mu_w[e,d]  = Wbar[d]*alpha[e,d] + beta[e,d],   Wbar = mean_h W
     var_w[e,d] = Vw[d]*alpha[e,d]^2,               Vw   = var_h W   (ddof=0)
  mu  = x @ mu_w.T + mean(bias); sig = sqrt(x^2 @ var_w.T + 1e-8)
  logits = erf(mu / (sqrt2*sig)); top-2 softmax -> router weights w1,w2
  out = sum_k w_k * relu(x @ (W*alpha[e_k]).T + bias)

Device strategy (data parallel over tokens, 512 tokens/core):
  - Router matmuls in fp32 (tiny: [128,1024]x[1024,8]).
  - Top-2 per token via the DVE Max8 sort; one-hots via is_equal vs v1/v2.
  - Per-token selected alpha row built with a tiny matmul over the 8-wide
    expert axis: A_k^T = alpha^T @ onehot_k^T, then xe = x * A_k (so only
    K=2 of the E=8 expert GEMMs are ever computed; no gather/scatter).
  - Main GEMMs in bf16 at N=512 against streamed W^T blocks; bias is added
    by a rank-1 ones x bias matmul into the same PSUM accumulation group;
    relu and the router weight are fused on the Scalar engine using
    relu(w*p) = w*relu(p) (w>0 as a softmax weight).
"""

import numpy as np
from contextlib import ExitStack

import os
import sys

if "/opt/trn_rl_repo" not in sys.path:
    sys.path.insert(0, "/opt/trn_rl_repo")

import ml_dtypes
import concourse.bass as bass
import concourse.tile as tile
from concourse import bacc, mybir
from concourse.bass_utils import run_bass_kernel_spmd

FP32 = mybir.dt.float32
BF16 = mybir.dt.bfloat16
AF = mybir.ActivationFunctionType
ALU = mybir.AluOpType

B, S, D, H, E = 2, 2048, 1024, 4096, 8
NCORES = 8
T = (B * S) // NCORES          # 512 tokens per core
NT = T // 128                  # 4 token tiles per core
DC = D // 128                  # 8 contraction chunks
HC = H // 512                  # 8 output column chunks
TG = T // 256                  # 2 selection token groups (N=256 keeps PE fast)


def _emit(ctx: ExitStack, tc: tile.TileContext, io: dict):
    nc = tc.nc
    xt, wt, alpha = io["xt"], io["wt"], io["alpha"]
    muw, varw = io["muw"], io["varw"]
    mb, nbias = io["mb"], io["nbias"]
    out = io["out"]

    const = ctx.enter_context(tc.tile_pool(name="const", bufs=1))
    persist = ctx.enter_context(tc.tile_pool(name="persist", bufs=1))

    # ---- small constant/parameter tiles ----
    muw_sb = const.tile([128, E * DC], FP32, name="muw_sb")
    varw_sb = const.tile([128, E * DC], FP32, name="varw_sb")
    mb_sb = const.tile([128, 1], FP32, name="mb_sb")
    alpha_sb = const.tile([E, D], BF16, name="alpha_sb")
    nbias_sb = const.tile([1, H], BF16, name="nbias_sb")
    ones_sb = const.tile([1, 128], BF16, name="ones_sb")
    ident_sb = const.tile([128, 128], FP32, name="ident_sb")
    eps_sb = const.tile([128, 1], FP32, name="eps_sb")
    nc.vector.memset(eps_sb[:], 2e-8)

    nc.sync.dma_start(muw_sb[:], muw[:])
    nc.sync.dma_start(varw_sb[:], varw[:])
    nc.sync.dma_start(mb_sb[:], mb[:])
    nc.sync.dma_start(alpha_sb[:], alpha[:])
    nc.sync.dma_start(nbias_sb[:], nbias[:])
    nc.vector.memset(ones_sb[:], 1.0)
    # identity for PE-transpose: keep ones where (p - f) == 0
    nc.vector.memset(ident_sb[:], 1.0)
    nc.gpsimd.affine_select(
        ident_sb[:], ident_sb[:], pattern=[[-1, 128]], base=0,
        channel_multiplier=1, compare_op=ALU.is_equal, fill=0.0,
    )

    # ---- x^T tiles (each a fully contiguous 256KB DRAM block) + x^2 ----
    xt_sb = []
    x2_sb = []
    for c in range(DC):
        t_ = persist.tile([128, T], FP32, name=f"xt{c}", tag=f"xt{c}")
        nc.sync.dma_start(t_[:], xt[128 * c:128 * (c + 1), :])
        xt_sb.append(t_)
    for c in range(DC):
        t_ = persist.tile([128, T], FP32, name=f"x2{c}", tag=f"x2{c}")
        nc.scalar.activation(t_[:], xt_sb[c][:], AF.Square)
        x2_sb.append(t_)

    mu_w = [muw_sb[:, E * c:E * (c + 1)] for c in range(DC)]
    var_w = [varw_sb[:, E * c:E * (c + 1)] for c in range(DC)]

    # ---- W^T fully resident: 8 big tiles [128, 4096] bf16 (8KB rows keep
    # the DMA descriptor count low; matmuls slice out [128, 512] columns) ----
    wt_sb = []
    for c in range(DC):
        w_ = persist.tile([128, H], BF16, name=f"wt{c}", tag=f"wt{c}")
        nc.sync.dma_start(w_[:], wt[128 * c:128 * (c + 1), :])
        wt_sb.append(w_)

    # ---- router + selection in two 256-token halves: the second half's
    # matmuls fill the first half's scalar-chain PE gap, and main GEMMs of
    # half 0 fill half 1's chain gap ----
    GT = 256
    xe = [[[None] * TG for _ in range(DC)] for _ in range(2)]
    for k in range(2):
        for c in range(DC):
            for g in range(TG):
                xe[k][c][g] = persist.tile([128, GT], BF16, name=f"xe{k}_{c}_{g}",
                                           tag=f"xe{k}_{c}_{g}")
    w_all = [None] * NT
    nb_sb = []
    sbuf_out = ctx.enter_context(tc.tile_pool(name="sbuf_out", bufs=1))
    spb = ctx.enter_context(tc.tile_pool(name="spb", bufs=2, space="PSUM"))
    spa = ctx.enter_context(tc.tile_pool(name="spa", bufs=2, space="PSUM"))

    def emit_router(g):
        gsl = slice(GT * g, GT * (g + 1))
        muT = spa.tile([E, GT], FP32, name=f"muT{g}", tag="spa")
        for c in range(DC):
            nc.tensor.matmul(muT[:], lhsT=mu_w[c], rhs=xt_sb[c][:, gsl],
                             start=(c == 0), stop=(c == DC - 1))
        margT = persist.tile([E, GT], FP32, name=f"margT{g}", tag=f"margT{g}")
        nc.vector.tensor_scalar_add(margT[:], muT[:], mb_sb[0:E, 0:1])
        vaT = spa.tile([E, GT], FP32, name=f"vaT{g}", tag="spa")
        for c in range(DC):
            nc.tensor.matmul(vaT[:], lhsT=var_w[c], rhs=x2_sb[c][:, gsl],
                             start=(c == 0), stop=(c == DC - 1))
        if g == 0:
            # -bias rows broadcast across partitions (fills the chain gap)
            for j in range(HC):
                nps = spb.tile([128, 512], FP32, name=f"nb_ps{j}", tag="spb")
                nc.tensor.matmul(nps[:], lhsT=ones_sb[:],
                                 rhs=nbias_sb[:, 512 * j:512 * (j + 1)],
                                 start=True, stop=True)
                nb_ = persist.tile([128, 512], BF16, name=f"nb{j}", tag=f"nb{j}")
                nc.vector.tensor_copy(nb_[:], nps[:])
                nb_sb.append(nb_)
        # sqrt(2*var + 2e-8) = sqrt(2)*sigma
        sig2T = persist.tile([E, GT], FP32, name=f"sig2T{g}", tag=f"sig2T{g}")
        nc.scalar.activation(sig2T[:], vaT[:], AF.Sqrt, bias=eps_sb[0:E, 0:1],
                             scale=2.0)
        recT = persist.tile([E, GT], FP32, name=f"recT{g}", tag=f"recT{g}")
        rscr = persist.tile([E, GT], FP32, name=f"rscr{g}", tag=f"rscr{g}")
        nc.vector.reciprocal_approx_accurate(recT[:], sig2T[:], rscr[:])
        logT = persist.tile([E, GT], FP32, name=f"logT{g}", tag=f"logT{g}")
        nc.vector.tensor_tensor(logT[:], margT[:], recT[:], op=ALU.mult)
        nc.scalar.activation(logT[:], logT[:], AF.Erf)
        return logT

    def emit_topk_sel(g, logT):
        gsl = slice(GT * g, GT * (g + 1))
        ohT = [None, None]
        for k in range(2):
            ohT[k] = persist.tile([E, GT], BF16, name=f"ohT{k}_{g}",
                                  tag=f"ohT{k}_{g}")
        for hh in range(2):
            ti = 2 * g + hh
            hsl = slice(128 * hh, 128 * (hh + 1))
            lg_ps = spa.tile([128, E], FP32, name=f"lg_ps{ti}", tag="spa")
            nc.tensor.transpose(lg_ps[:], logT[:, hsl], ident_sb[0:E, 0:E])
            lg = persist.tile([128, E], FP32, name=f"lg{ti}", tag=f"lg{ti}")
            nc.vector.tensor_copy(lg[:], lg_ps[:])
            mx = persist.tile([128, 8], FP32, name=f"mx{ti}", tag=f"mx{ti}")
            nc.vector.max(mx[:], lg[:])
            o1 = persist.tile([128, E], FP32, name=f"oh1_{ti}", tag=f"oh1_{ti}")
            nc.vector.tensor_scalar(o1[:], lg[:], mx[:, 0:1], None,
                                    op0=ALU.is_equal)
            o2 = persist.tile([128, E], FP32, name=f"oh2_{ti}", tag=f"oh2_{ti}")
            nc.vector.tensor_scalar(o2[:], lg[:], mx[:, 1:2], None,
                                    op0=ALU.is_equal)
            d_ = persist.tile([128, 1], FP32, name=f"d21_{ti}", tag=f"d21_{ti}")
            nc.vector.tensor_tensor(d_[:], mx[:, 0:1], mx[:, 1:2],
                                    op=ALU.subtract)
            w_ = persist.tile([128, 2], FP32, name=f"w{ti}", tag=f"w{ti}")
            nc.scalar.activation(w_[:, 0:1], d_[:], AF.Sigmoid)
            nc.vector.tensor_scalar(w_[:, 1:2], w_[:, 0:1], -1.0, 1.0,
                                    op0=ALU.mult, op1=ALU.add)
            w_all[ti] = w_
            for k, o_ in ((0, o1), (1, o2)):
                tp = spb.tile([E, 128], FP32, name=f"ohTp{k}_{ti}", tag="spb")
                nc.tensor.transpose(tp[:], o_[:], ident_sb[:])
                nc.vector.tensor_copy(ohT[k][:, hsl], tp[:])
        # selection: xe[k][c][g] = x * alpha[e_k(t)]  (bf16, [d, t] layout)
        for c in range(DC):
            for k in range(2):
                a_ps = spb.tile([128, GT], FP32, name=f"a_ps{g}{k}{c}", tag="spb")
                nc.tensor.matmul(a_ps[:], lhsT=alpha_sb[:, 128 * c:128 * (c + 1)],
                                 rhs=ohT[k][:], start=True, stop=True)
                nc.vector.tensor_tensor(xe[k][c][g][:], xt_sb[c][:, gsl],
                                        a_ps[:], op=ALU.mult)

    # ---- main GEMMs: JQ h-chunks of 512 share one stationary load ----
    JQ = 2
    ps_main = ctx.enter_context(tc.tile_pool(name="ps_main", bufs=4, space="PSUM"))

    def emit_main(ti, jq, js=None):
        js = list(range(jq * JQ, (jq + 1) * JQ)) if js is None else js
        tsl = slice(128 * ti, 128 * (ti + 1))
        g, hh = ti // 2, ti % 2
        hsl = slice(128 * hh, 128 * (hh + 1))
        s_tiles = [[None] * len(js), [None] * len(js)]
        for k in range(2):
            ps = [ps_main.tile([128, 512], FP32, name=f"ps{jq}_{ti}_{k}_{jj}",
                               tag="ps_main") for jj in range(len(js))]
            for c in range(DC):
                for jj, j in enumerate(js):
                    nc.tensor.matmul(ps[jj][:], lhsT=xe[k][c][g][:, hsl],
                                     rhs=wt_sb[c][:, 512 * j:512 * (j + 1)],
                                     start=(c == 0), stop=(c == DC - 1))
            for jj, j in enumerate(js):
                # relu(p + b) == max(p, -b) + b; the +b lands after combine
                # (w1 + w2 == 1 exactly: w1 >= 0.5 so 1 - w1 is Sterbenz-exact)
                m_ = sbuf_out.tile([128, 512], FP32, name=f"m{jq}_{ti}_{k}_{jj}",
                                   tag=f"s{k}", bufs=JQ + 2)
                nc.vector.tensor_tensor(m_[:], ps[jj][:], nb_sb[j][:], op=ALU.max)
                # w_k * max(p, -b) on the Scalar engine (Copy with scale AP)
                nc.scalar.activation(m_[:], m_[:], AF.Copy,
                                     scale=w_all[ti][:, k:k + 1])
                s_tiles[k][jj] = m_
        o_ = sbuf_out.tile([128, 512 * len(js)], FP32, name=f"o{jq}_{ti}",
                           tag="otile", bufs=4)
        for jj, j in enumerate(js):
            u_ = sbuf_out.tile([128, 512], FP32, name=f"u{jq}_{ti}_{jj}",
                               tag="utile", bufs=4)
            nc.vector.tensor_tensor(u_[:], s_tiles[0][jj][:], s_tiles[1][jj][:],
                                    op=ALU.add)
            nc.vector.tensor_tensor(o_[:, 512 * jj:512 * (jj + 1)],
                                    u_[:], nb_sb[j][:], op=ALU.subtract)
        nc.sync.dma_start(out[tsl, 512 * js[0]:512 * (js[-1] + 1)], o_[:])

    # emission order: g0 routing, B(t0), then g1 routing (fills B(t0)'s
    # stalls and its scalar chain overlaps B), then the remaining tiles
    logT = emit_router(0)
    emit_topk_sel(0, logT)
    logT1 = emit_router(1)
    for jq in range(HC // JQ):
        emit_main(0, jq)
    emit_topk_sel(1, logT1)
    for ti in range(1, NT):
        for jq in range(HC // JQ):
            if ti == NT - 1 and jq == HC // JQ - 1:
                # split the final group so its epilogue overlaps compute
                emit_main(ti, jq, js=[HC - 2])
                emit_main(ti, jq + 1, js=[HC - 1])
            else:
                emit_main(ti, jq)


_CACHE = {}

if os.environ.get("BASS_LDW_OPT") == "1":
    import concourse.bass_utils as _bu

    _orig_run_command = _bu.run_command

    def _run_command_ldw(cmd, *a, **kw):
        cmd = ["--enable-ldw-opt=true" if c == "--enable-ldw-opt=false" else c
               for c in cmd]
        return _orig_run_command(cmd, *a, **kw)

    _bu.run_command = _run_command_ldw


def _build():
    if "nc" in _CACHE:
        return _CACHE["nc"]
    nc = bacc.Bacc("TRN2", target_bir_lowering=False, debug=False,
                   num_devices=NCORES)
    io = {
        "xt": nc.dram_tensor("xt", [D, T], FP32, kind="ExternalInput").ap(),
        "wt": nc.dram_tensor("wt", [D, H], BF16, kind="ExternalInput").ap(),
        "alpha": nc.dram_tensor("alpha", [E, D], BF16, kind="ExternalInput").ap(),
        "muw": nc.dram_tensor("muw", [128, E * DC], FP32,
                              kind="ExternalInput").ap(),
        "varw": nc.dram_tensor("varw", [128, E * DC], FP32,
                               kind="ExternalInput").ap(),
        "mb": nc.dram_tensor("mb", [128, 1], FP32, kind="ExternalInput").ap(),
        "nbias": nc.dram_tensor("nbias", [1, H], BF16, kind="ExternalInput").ap(),
        "out": nc.dram_tensor("out", [T, H], FP32, kind="ExternalOutput").ap(),
    }
    with tile.TileContext(nc) as tc, ExitStack() as ctx:
        _emit(ctx, tc, io)
    nc.compile()
    _CACHE["nc"] = nc
    return nc


def _chunk_cols(m):
    # [D, n] -> [128, DC*n] where columns [n*c : n*(c+1)] hold rows 128c..128c+127
    n = m.shape[1]
    return np.ascontiguousarray(
        m.reshape(DC, 128, n).transpose(1, 0, 2).reshape(128, DC * n))


def make_in_maps(x, W, bias, alpha, beta):
    tokens = np.ascontiguousarray(x.reshape(B * S, D))
    Wbar = W.mean(axis=0).astype(np.float32)
    Vw = W.var(axis=0).astype(np.float32)
    mu_w = (Wbar[None, :] * alpha + beta).astype(np.float32)    # [E, D]
    var_w = (Vw[None, :] * alpha * alpha).astype(np.float32)    # [E, D]
    mb = np.full((128, 1), bias.mean(), dtype=np.float32)
    wt_bf = np.ascontiguousarray(W.T).astype(ml_dtypes.bfloat16)
    muw_c = _chunk_cols(np.ascontiguousarray(mu_w.T))
    varw_c = _chunk_cols(np.ascontiguousarray(var_w.T))
    nbias = (-bias).reshape(1, H).astype(ml_dtypes.bfloat16)
    common = dict(wt=wt_bf, alpha=np.ascontiguousarray(alpha).astype(ml_dtypes.bfloat16),
                  muw=muw_c, varw=varw_c, mb=mb, nbias=nbias)
    maps = []
    for m in range(NCORES):
        xs = np.ascontiguousarray(tokens[T * m:T * (m + 1)].T.astype(np.float32))
        maps.append(dict(xt=xs, **common))
    return maps


def run(x, W, bias, alpha, beta, trace=False, **kw):
    nc = _build()
    maps = make_in_maps(x, W, bias, alpha, beta)
    res = run_bass_kernel_spmd(nc, maps, core_ids=list(range(NCORES)),
                               trace=trace, **kw)
    outs = [res.results[m]["out"] for m in range(NCORES)]
    full = np.concatenate(outs, axis=0).reshape(B, S, H).astype(np.float32)
    return full, res


def kernel(x, W, bias, alpha, beta):
    full, _ = run(np.asarray(x), np.asarray(W), np.asarray(bias),
                  np.asarray(alpha), np.asarray(beta))
    return full
